# revision 33
# baseline (speedup 1.0000x reference)
"""Multi-head Koopman module on 8 Trainium2 NeuronCores.

Math: out_k^T = E_k Q_k^T with E_k = gate_k * B_v L A^2 L^{-1}  (per b,h),
so   y_b = sg * normed_b @ W_all_b,  W_all_b = sum_{k,h} Wq_{k,h} E^T W_{O,h}.
The queries never need to be materialized on device: phase 1 computes the
prefix Gram statistics (G, M, Cv) per (k, head) plus the centered/transposed
activations; the host does the 48x48 cholesky/solve/SVD algebra and folds
everything into a per-batch [D, D] effective matrix; phase 2 is one big
bf16 matmul  y = rstd * (Xc @ W_eff).

Sharding: core c -> batch b = c//2, head-half hh = c%2 (8 of 16 heads) for
phase 1; same core -> (batch, token-half) for phase 2.

All device matmuls run in bf16 (1 cycle/row on the PE vs 4 for fp32);
accumulation stays fp32 in PSUM. LN is folded: x is centered (exact, f32
stats) and cast to bf16 before the PE transpose; rstd is applied in the
projection epilogues; gamma is folded into the weights on the host.

Phase-1 schedule keeps the PE continuously fed (p-state ramps to 2.4 GHz
after 3 us of uninterrupted work): per prefix tile the transpose of tile
it+1 is interleaved with the projections of tile it; suffix-tile (non-
prefix) preprocessing runs after, overlapped with the shift/gram stages.
Shifted keys for the cross-covariance M are produced on the PE via a
shift matrix (SBUF->SBUF DMA serializes on one engine; partition-offset
matmul operands are illegal), and written back into the kvs tile so each
gram pair is a single [96, 256] matmul chain: [G | Cv | M'].
"""

import math

import numpy as np
import ml_dtypes

B, T, D = 4, 2048, 1024
H, HD = 16, 64
K_OPS, R = 4, 48
LN_EPS = 1e-5
NCORES = 8
HPC = H // 2            # heads per core = 8
NKQ = HPC * R           # 384 (per-core K width per op)
NV = HPC * HD           # 512
ND = D // 128           # 8 d-tiles
NTT = T // 128          # 16 token tiles
TH = T // 2             # phase-2 token half
# kvs per-head slot layout (width 512):
# [k0(0:48) k1(48:96) V(96:160) sh01(160:256) k2(256:304) k3(304:352)
#  Vd(352:416) sh23(416:512)]
KSLOT = [0, 48, 256, 304]
HW = 512

BF16 = ml_dtypes.bfloat16

_cache = {}


def _split_multi_waits(nc):
    """walrus codegen accepts at most one sync wait per instruction;
    move extra waits onto preceding wait-only NoOps on the same engine."""
    from concourse import mybir
    for fn in nc.m.functions:
        for bb in fn.blocks:
            insts = list(bb.instructions)
            new = []
            changed = False
            for inst in insts:
                si = inst.sync_info
                if si is not None and si.on_wait and len(si.on_wait) > 1:
                    waits = list(si.on_wait)
                    for j, w in enumerate(waits[:-1]):
                        new.append(mybir.InstNoOp(
                            name=f"{inst.name}-ws{j}", engine=inst.engine,
                            ins=[], outs=[],
                            sync_info=mybir.SyncInfo(on_wait=[w], on_update=[])))
                    inst.sync_info = mybir.SyncInfo(on_wait=[waits[-1]],
                                                    on_update=list(si.on_update))
                    changed = True
                new.append(inst)
            if changed:
                bb.instructions = new
    return nc


def _build_phase1(pl: int):
    import concourse.bass as bass
    import concourse.tile as tile
    from concourse import mybir
    from concourse.masks import make_identity
    from contextlib import ExitStack

    f32 = mybir.dt.float32
    bf16 = mybir.dt.bfloat16
    nc = bass.Bass()

    n_pt = (pl + 127) // 128     # prefix tiles (pl <= T-1 so n_pt <= NTT)
    nb = n_pt - 1                # tile-boundary count for the shifted gram

    xb = nc.dram_tensor("xb", [T, D], f32, kind="ExternalInput")
    wk = nc.dram_tensor("wk", [K_OPS, D, NKQ], bf16, kind="ExternalInput")
    wv = nc.dram_tensor("wv", [D, NV], bf16, kind="ExternalInput")
    xct_out = nc.dram_tensor("xct_out", [ND, 128, T], bf16, kind="ExternalOutput")
    rstd_out = nc.dram_tensor("rstd_out", [128, NTT], f32, kind="ExternalOutput")
    gmc_out = nc.dram_tensor("gmc_out", [HPC, 96, 512], f32, kind="ExternalOutput")

    with tile.TileContext(nc) as tc, ExitStack() as ctx:
        const = ctx.enter_context(tc.tile_pool(name="const", bufs=1))
        xch = ctx.enter_context(tc.tile_pool(name="xch", bufs=2))
        wkp = ctx.enter_context(tc.tile_pool(name="wkp", bufs=1))
        xctp = ctx.enter_context(tc.tile_pool(name="xctp", bufs=1))
        kvsp = ctx.enter_context(tc.tile_pool(name="kvsp", bufs=1))
        xcp = ctx.enter_context(tc.tile_pool(name="xcp", bufs=3))
        junkp = ctx.enter_context(tc.tile_pool(name="junkp", bufs=2))
        gstp = ctx.enter_context(tc.tile_pool(name="gstp", bufs=2))

        ident = const.tile([128, 128], bf16)
        make_identity(nc, ident)
        # shift matrix: S[t, j] = 1 iff t == j + 1, so (S^T K)[j] = K[j+1]
        shmat = const.tile([128, 128], bf16)
        nc.gpsimd.memset(shmat, 0.0)
        nc.gpsimd.affine_select(out=shmat, in_=shmat,
                                compare_op=mybir.AluOpType.not_equal,
                                fill=1.0, base=-1, channel_multiplier=1,
                                pattern=[[-1, 128]])
        eps_t = const.tile([128, 1], f32)
        nc.vector.memset(eps_t, LN_EPS)

        s_all = const.tile([128, NTT], f32)
        q_all = const.tile([128, NTT], f32)
        mneg_all = const.tile([128, NTT], f32)
        var_all = const.tile([128, NTT], f32)
        std_all = const.tile([128, NTT], f32)
        rstd_all = const.tile([128, NTT], f32)
        t1_all = const.tile([128, NTT], f32)

        # xcT in halves so the low half can stream out to DRAM early
        xcT_lo = xctp.tile([128, ND, TH], bf16, tag="lo")
        xcT_hi = xctp.tile([128, ND, TH], bf16, tag="hi")

        def xcT(it):
            # (tile, column slice) for token tile it
            half, off = divmod(it, 8)
            t = xcT_lo if half == 0 else xcT_hi
            return t, slice(off * 128, (off + 1) * 128)

        kvs = kvsp.tile([128, n_pt, HPC, HW], bf16)

        # x arrives in 2-tile chunks so work starts after the first 1 MB
        xchunks = {}

        def load_chunk(c):
            xt = xch.tile([128, 2, D], f32, tag="xch")
            nc.sync.dma_start(
                out=xt,
                in_=xb[c * 256:(c + 1) * 256, :].rearrange("(a p) n -> p a n", p=128))
            xchunks[c] = xt

        def prep_tile(it, tp_ps):
            if it // 2 not in xchunks:
                load_chunk(it // 2)
            xt = xchunks[it // 2][:, it % 2, :]
            c0, c1 = it, it + 1
            junk = junkp.tile([128, D], bf16)
            nc.scalar.activation(out=junk, in_=xt,
                                 func=mybir.ActivationFunctionType.Square,
                                 accum_out=q_all[:, c0:c1])
            nc.vector.tensor_reduce(out=s_all[:, c0:c1], in_=xt,
                                    axis=mybir.AxisListType.X,
                                    op=mybir.AluOpType.add)
            nc.vector.tensor_scalar_mul(mneg_all[:, c0:c1], s_all[:, c0:c1],
                                        -1.0 / D)
            nc.vector.tensor_mul(t1_all[:, c0:c1], s_all[:, c0:c1],
                                 s_all[:, c0:c1])
            nc.vector.tensor_scalar_mul(t1_all[:, c0:c1], t1_all[:, c0:c1],
                                        1.0 / D)
            nc.vector.tensor_sub(var_all[:, c0:c1], q_all[:, c0:c1],
                                 t1_all[:, c0:c1])
            nc.scalar.activation(out=std_all[:, c0:c1], in_=var_all[:, c0:c1],
                                 func=mybir.ActivationFunctionType.Sqrt,
                                 bias=eps_t[:, 0:1], scale=1.0 / D)
            nc.vector.reciprocal(rstd_all[:, c0:c1], std_all[:, c0:c1])
            xc = xcp.tile([128, D], bf16, tag="xc")
            nc.scalar.activation(out=xc, in_=xt,
                                 func=mybir.ActivationFunctionType.Identity,
                                 bias=mneg_all[:, c0:c1], scale=1.0)
            tp = tp_ps.tile([128, D], bf16)
            for d in range(ND):
                nc.tensor.transpose(tp[:, d * 128:(d + 1) * 128],
                                    xc[:, d * 128:(d + 1) * 128], ident)
            xt_t, xt_sl = xcT(it)
            nc.vector.tensor_copy(
                out=xt_t[:, :, xt_sl],
                in_=tp.rearrange("p (a n) -> p a n", a=ND))

        def proj_tile(it, proj_ps, vproj_ps):
            xt_t, tsl = xcT(it)
            for k in range(K_OPS):
                kp = proj_ps.tile([128, NKQ], f32)
                for d in range(ND):
                    nc.tensor.matmul(kp, xt_t[:, d, tsl],
                                     wk_sb[:, d, k * NKQ:(k + 1) * NKQ],
                                     start=(d == 0), stop=(d == ND - 1))
                ks = KSLOT[k]
                nc.scalar.activation(
                    out=kvs[:, it, :, ks:ks + R],
                    in_=kp.rearrange("p (h r) -> p h r", h=HPC),
                    func=mybir.ActivationFunctionType.Copy,
                    bias=0.0, scale=rstd_all[:, it:it + 1])
            vp = vproj_ps.tile([128, NV], f32)
            for d in range(ND):
                nc.tensor.matmul(vp, xt_t[:, d, tsl], wv_sb[:, d, :],
                                 start=(d == 0), stop=(d == ND - 1))
            nc.scalar.activation(
                out=kvs[:, it, :, 96:160],
                in_=vp.rearrange("p (h v) -> p h v", h=HPC),
                func=mybir.ActivationFunctionType.Copy,
                bias=0.0, scale=rstd_all[:, it:it + 1])
            # duplicate V slot for the pair23 gram pack (already scaled)
            nc.vector.tensor_copy(out=kvs[:, it, :, 352:416],
                                  in_=kvs[:, it, :, 96:160])
            if it == n_pt - 1 and pl < n_pt * 128:
                # zero keys/values of padded tokens so grams stay exact
                nc.vector.memset(kvs[pl - (n_pt - 1) * 128:128, it, :, :], 0.0)

        # first x chunk before the weights so tile-0 prep starts earliest
        load_chunk(0)
        wk_sb = wkp.tile([128, ND, K_OPS * NKQ], bf16)
        for k in range(K_OPS):
            nc.sync.dma_start(
                out=wk_sb[:, :, k * NKQ:(k + 1) * NKQ],
                in_=wk[k].rearrange("(a p) n -> p a n", p=128))
        wv_sb = wkp.tile([128, ND, NV], bf16)
        nc.sync.dma_start(out=wv_sb, in_=wv.rearrange("(a p) n -> p a n", p=128))

        # interleave prep(it) with proj(it-1) to keep the PE continuously fed
        with tc.tile_pool(name="tp_ps", bufs=2, space="PSUM") as tp_ps, \
             tc.tile_pool(name="warm_ps", bufs=1, space="PSUM") as warm_ps, \
             tc.tile_pool(name="proj_ps", bufs=2, space="PSUM") as proj_ps, \
             tc.tile_pool(name="vproj_ps", bufs=2, space="PSUM") as vproj_ps:
            # warm the PE p-state while the first x chunk is in flight
            warm_sb = const.tile([128, 512], bf16)
            nc.vector.memset(warm_sb, 0.0)
            wps = warm_ps.tile([128, 512], f32)
            for _ in range(16):
                nc.tensor.matmul(wps, ident, warm_sb, start=True, stop=True)
            for it in range(NTT):
                prep_tile(it, tp_ps)
                if 1 <= it <= n_pt:
                    proj_tile(it - 1, proj_ps, vproj_ps)
                if it == 7:
                    # low half of xcT is complete: stream it out early
                    for d in range(ND):
                        nc.sync.dma_start(out=xct_out[d, :, 0:TH],
                                          in_=xcT_lo[:, d, :])
            if n_pt == NTT:
                proj_tile(n_pt - 1, proj_ps, vproj_ps)

        # ---- shifted keys on the PE, written back into kvs sh-slots ----
        with tc.tile_pool(name="sh_ps", bufs=4, space="PSUM") as sh_ps:
            for it in range(n_pt):
                for g, (ssl, dsl) in enumerate(
                        [(slice(0, 96), slice(160, 256)),
                         (slice(256, 352), slice(416, 512))]):
                    for hf in range(2):
                        hsl = slice(hf * 4, (hf + 1) * 4)
                        sp = sh_ps.tile([128, 4 * 96], f32)
                        nc.tensor.matmul(
                            sp.rearrange("p (h n) -> p h n", h=4), shmat,
                            kvs[:, it, hsl, ssl], start=True, stop=True)
                        nc.vector.tensor_copy(
                            out=kvs[:, it, hsl, dsl],
                            in_=sp.rearrange("p (h n) -> p h n", h=4))

        # boundary rows for M: row 127 of tile it (B1) x row 0 of tile it+1
        if nb > 0:
            bp = ctx.enter_context(tc.tile_pool(name="bp", bufs=1))
            b1 = bp.tile([nb, HPC, 2, 96], bf16)
            b0 = bp.tile([nb, HPC, 2, 96], bf16)
            nc.sync.dma_start(out=b1[:, :, 0, :], in_=kvs[127:128, 0:nb, :, 0:96])
            nc.sync.dma_start(out=b1[:, :, 1, :], in_=kvs[127:128, 0:nb, :, 256:352])
            nc.sync.dma_start(out=b0[:, :, 0, :], in_=kvs[0:1, 1:n_pt, :, 0:96])
            nc.sync.dma_start(out=b0[:, :, 1, :], in_=kvs[0:1, 1:n_pt, :, 256:352])

        # ---- packed grams: one [96, 256] chain per k-pair ----
        # out cols [0:96] = G blocks, [96:160] = Cv^T, [160:256] = M'
        with tc.tile_pool(name="gram_ps", bufs=2, space="PSUM") as gram_ps:
            for h in range(HPC):
                psa = gram_ps.tile([96, 256], f32, tag="a")
                psb = gram_ps.tile([96, 256], f32, tag="b")
                for it in range(n_pt):
                    last = (it == n_pt - 1)
                    nc.tensor.matmul(psa, kvs[:, it, h, 0:96],
                                     kvs[:, it, h, 0:256],
                                     start=(it == 0), stop=(last and nb == 0))
                    nc.tensor.matmul(psb, kvs[:, it, h, 256:352],
                                     kvs[:, it, h, 256:512],
                                     start=(it == 0), stop=(last and nb == 0))
                if nb > 0:
                    nc.tensor.matmul(psa[:, 160:256], b1[:, h, 0, :],
                                     b0[:, h, 0, :], start=False, stop=True,
                                     skip_group_check=True)
                    nc.tensor.matmul(psb[:, 160:256], b1[:, h, 1, :],
                                     b0[:, h, 1, :], start=False, stop=True,
                                     skip_group_check=True)
                gst = gstp.tile([96, 512], f32, tag="gst")
                nc.vector.tensor_copy(out=gst[:, 0:256], in_=psa)
                nc.vector.tensor_copy(out=gst[:, 256:512], in_=psb)
                nc.sync.dma_start(out=gmc_out[h], in_=gst)

        nc.sync.dma_start(out=rstd_out[:, :], in_=rstd_all)
        for d in range(ND):
            nc.sync.dma_start(out=xct_out[d, :, TH:T], in_=xcT_hi[:, d, :])
    return _split_multi_waits(nc)


def _build_phase2():
    import concourse.bass as bass
    import concourse.tile as tile
    from concourse import mybir
    from contextlib import ExitStack

    f32 = mybir.dt.float32
    bf16 = mybir.dt.bfloat16
    nc = bass.Bass()
    xct = nc.dram_tensor("xct", [ND, 128, TH], bf16, kind="ExternalInput")
    weff = nc.dram_tensor("weff", [ND, 128, D], bf16, kind="ExternalInput")
    rstd = nc.dram_tensor("rstd", [128, TH // 128], f32, kind="ExternalInput")
    y_out = nc.dram_tensor("y_out", [TH // 128, 128, D], f32, kind="ExternalOutput")

    nth = TH // 128  # 8 token tiles

    with tile.TileContext(nc) as tc, ExitStack() as ctx:
        const = ctx.enter_context(tc.tile_pool(name="const", bufs=1))
        ystp = ctx.enter_context(tc.tile_pool(name="ystp", bufs=3))
        y_ps = ctx.enter_context(tc.tile_pool(name="y_ps", bufs=2, space="PSUM"))

        # per-d tiles so the first matmuls start as soon as d0 lands
        xct_d, weff_d = [], []
        for d in range(ND):
            xt = const.tile([128, TH], bf16, tag=f"x{d}")
            nc.sync.dma_start(out=xt, in_=xct[d])
            wt = const.tile([128, D], bf16, tag=f"w{d}")
            nc.sync.dma_start(out=wt, in_=weff[d])
            xct_d.append(xt)
            weff_d.append(wt)
        rstd_sb = const.tile([128, nth], f32)
        nc.sync.dma_start(out=rstd_sb, in_=rstd[:, :])

        # warm the PE p-state while the loads are in flight
        warm_sb = const.tile([128, 512], bf16)
        nc.vector.memset(warm_sb, 0.0)
        with tc.tile_pool(name="warm_ps", bufs=1, space="PSUM") as warm_ps:
            wps = warm_ps.tile([128, 512], f32)
            for _ in range(24):
                nc.tensor.matmul(wps, warm_sb[:, 0:128], warm_sb,
                                 start=True, stop=True)

        for tt in range(nth):
            tsl = slice(tt * 128, (tt + 1) * 128)
            yp = y_ps.tile([128, D], f32)
            for d in range(ND):
                nc.tensor.matmul(yp[:, 0:512], xct_d[d][:, tsl],
                                 weff_d[d][:, 0:512],
                                 start=(d == 0), stop=(d == ND - 1))
                nc.tensor.matmul(yp[:, 512:1024], xct_d[d][:, tsl],
                                 weff_d[d][:, 512:1024],
                                 start=(d == 0), stop=(d == ND - 1))
            y_sb = ystp.tile([128, D], f32, tag="y_sb")
            nc.scalar.activation(out=y_sb, in_=yp,
                                 func=mybir.ActivationFunctionType.Copy,
                                 bias=0.0, scale=rstd_sb[:, tt:tt + 1])
            nc.gpsimd.dma_start(out=y_out[tt], in_=y_sb)
    return _split_multi_waits(nc)


LAST_PERF = {}


def _numpy_fallback(hidden_states, W_K_ops, W_Q_ops, W_V, W_O, ln_gamma, ln_beta,
                    gate_alphas, gate_alpha, log_ridges, log_gammas, pl):
    x = np.asarray(hidden_states, np.float64)
    mu = x.mean(-1, keepdims=True)
    var = x.var(-1, keepdims=True)
    normed = (x - mu) / np.sqrt(var + LN_EPS) * ln_gamma + ln_beta
    values = (normed @ W_V).reshape(B, T, H, HD).transpose(0, 2, 1, 3)
    acc = np.zeros((B, H, T, HD))
    eye = np.eye(R)
    for k in range(K_OPS):
        ridge = math.exp(float(log_ridges[k]))
        gamma = math.exp(float(log_gammas[k]))
        gate = 1.0 / (1.0 + math.exp(-float(gate_alphas[k])))
        keys = (normed @ W_K_ops[k]).reshape(B, T, H, R).transpose(0, 2, 1, 3)
        qs = (normed @ W_Q_ops[k]).reshape(B, T, H, R).transpose(0, 2, 1, 3)
        pk = keys[:, :, :pl, :]
        G = np.einsum('bhlr,bhls->bhrs', pk, pk) + ridge * eye
        M = np.einsum('bhlr,bhls->bhrs', pk[:, :, 1:, :], pk[:, :, :-1, :])
        L = np.linalg.cholesky(G)
        Linv = np.linalg.inv(L)
        A = Linv @ M @ np.swapaxes(Linv, -1, -2)
        sig = np.linalg.svd(A, compute_uv=False)[..., 0]
        sig = np.maximum(sig, 1e-8)
        scale = min(gamma, 1.0) / np.maximum(sig, 1.0)
        A = A * scale[..., None, None]
        pv = values[:, :, :pl, :]
        Cv = np.einsum('bhld,bhlr->bhdr', pv, pk)
        Ginv = np.swapaxes(Linv, -1, -2) @ Linv
        Bv = Cv @ Ginv
        E = Bv @ L @ A @ A @ Linv
        out_k = np.einsum('bhdr,bhtr->bhtd', E, qs)
        acc = acc + gate * out_k
    out = acc.transpose(0, 2, 1, 3).reshape(B, T, H * HD) @ W_O
    sg = 1.0 / (1.0 + math.exp(-float(np.asarray(gate_alpha).ravel()[0])))
    return (sg * out).astype(np.float32)


def _decode_gmc(blk, k):
    """blk: [96, 512] device gram pack for one head; returns (G, CvT, Mp)."""
    pr, sub = divmod(k, 2)
    base = 256 * pr
    rsl = slice(sub * R, (sub + 1) * R)
    G = blk[rsl, base + sub * R:base + (sub + 1) * R]
    CvT = blk[rsl, base + 96:base + 160]
    Mp = blk[rsl, base + 160 + sub * R:base + 160 + (sub + 1) * R]
    return G, CvT, Mp


def kernel(hidden_states, W_K_ops, W_Q_ops, W_V, W_O, ln_gamma, ln_beta,
           gate_alphas, gate_alpha, log_ridges, log_gammas, prefix_len):
    from concourse.bass_utils import run_bass_kernel_spmd

    hidden_states = np.ascontiguousarray(np.asarray(hidden_states, np.float32))
    W_K_ops = np.asarray(W_K_ops, np.float32)
    W_Q_ops = np.asarray(W_Q_ops, np.float32)
    W_V = np.asarray(W_V, np.float32)
    W_O = np.asarray(W_O, np.float32)
    ln_gamma = np.asarray(ln_gamma, np.float32)
    ln_beta = np.asarray(ln_beta, np.float32)
    gate_alphas = np.asarray(gate_alphas, np.float32)
    log_ridges = np.asarray(log_ridges, np.float32)
    log_gammas = np.asarray(log_gammas, np.float32)
    pl = max(1, min(int(prefix_len), T - 1))

    if np.any(ln_beta != 0) or pl < 2:
        return _numpy_fallback(hidden_states, W_K_ops, W_Q_ops, W_V, W_O,
                               ln_gamma, ln_beta, gate_alphas, gate_alpha,
                               log_ridges, log_gammas, pl)

    # fold LN gamma into the projection weights; cast to bf16 for the device
    wk_f = (W_K_ops * ln_gamma[None, :, None]).astype(BF16)
    wv_f = (W_V * ln_gamma[:, None]).astype(BF16)

    wk_arr, wv_arr = [], []
    for hh in range(2):
        h0 = hh * HPC
        wk_arr.append(np.ascontiguousarray(wk_f[:, :, h0 * R:(h0 + HPC) * R]))
        wv_arr.append(np.ascontiguousarray(wv_f[:, h0 * HD:(h0 + HPC) * HD]))
    in1 = [{"xb": hidden_states[c // 2],
            "wk": wk_arr[c % 2], "wv": wv_arr[c % 2]} for c in range(NCORES)]

    key1 = ("p1", pl)
    if key1 not in _cache:
        _cache[key1] = _build_phase1(pl)
    r1 = run_bass_kernel_spmd(_cache[key1], in1, core_ids=list(range(NCORES)))
    LAST_PERF["p1"] = r1

    # ---- host: unpack G/M/Cv, 48x48 algebra, fold into per-batch W_eff ----
    ridge = np.exp(log_ridges.astype(np.float64))
    gamma_k = np.exp(log_gammas.astype(np.float64))
    gates = 1.0 / (1.0 + np.exp(-gate_alphas.astype(np.float64)))
    sg = 1.0 / (1.0 + math.exp(-float(np.asarray(gate_alpha).ravel()[0])))
    eye = np.eye(R)

    # E[b, k, h, HD, R]
    E = np.empty((B, K_OPS, H, HD, R), np.float64)
    for c in range(NCORES):
        b, h0 = c // 2, (c % 2) * HPC
        gmc = r1.results[c]["gmc_out"].astype(np.float64)  # [HPC, 96, 512]
        for hh in range(HPC):
            for k in range(K_OPS):
                G, CvT, Mp = _decode_gmc(gmc[hh], k)
                Gk = G + ridge[k] * eye
                M = Mp.T                      # M' = sum k_t k_{t+1}^T
                Cv = CvT.T                    # [HD, R]
                L = np.linalg.cholesky(Gk)
                Linv = np.linalg.inv(L)
                A = Linv @ M @ Linv.T
                sig = np.linalg.svd(A, compute_uv=False)[0]
                sig = max(sig, 1e-8)
                scale = min(gamma_k[k], 1.0) / max(sig, 1.0)
                A = A * scale
                Ginv = Linv.T @ Linv
                Bv = Cv @ Ginv
                E[b, k, h0 + hh] = gates[k] * (Bv @ L @ A @ A @ Linv)

    # W_eff[b] = sum_{k,h} (gamma o Wq_{k,h}) @ (E^T @ Wo_h), then * sg
    wq_f = (W_Q_ops * ln_gamma[None, :, None]).astype(np.float32)
    wq_flat = np.ascontiguousarray(
        wq_f.transpose(1, 0, 2).reshape(D, K_OPS * H * R))
    wo_r = W_O.reshape(H, HD, D)
    Et = np.ascontiguousarray(E.transpose(0, 1, 2, 4, 3).astype(np.float32))
    T1 = Et @ wo_r[None, None]               # [B, K, H, R, D]
    T1_flat = T1.reshape(B, K_OPS * H * R, D)
    weff_b = (wq_flat[None] @ T1_flat) * np.float32(sg)   # [B, D, D]

    if "p2" not in _cache:
        _cache["p2"] = _build_phase2()
    in2 = []
    for c in range(NCORES):
        b, hh = c // 2, c % 2
        xct_full = r1.results[c]["xct_out"]   # [ND, 128, T] bf16
        in2.append({
            "xct": np.ascontiguousarray(xct_full[:, :, hh * TH:(hh + 1) * TH]),
            "weff": np.ascontiguousarray(
                weff_b[b].astype(BF16).reshape(ND, 128, D)),
            "rstd": np.ascontiguousarray(
                r1.results[c]["rstd_out"][:, hh * 8:(hh + 1) * 8]),
        })
    r2 = run_bass_kernel_spmd(_cache["p2"], in2, core_ids=list(range(NCORES)))
    LAST_PERF["p2"] = r2

    y = np.empty((B, T, D), np.float32)
    for b in range(B):
        y[b, :TH] = r2.results[2 * b]["y_out"].reshape(TH, D)
        y[b, TH:] = r2.results[2 * b + 1]["y_out"].reshape(TH, D)
    return y


# revision 35
# speedup vs baseline: 1.0083x; 1.0083x over previous
"""Multi-head Koopman module on 8 Trainium2 NeuronCores.

Math: out_k^T = E_k Q_k^T with E_k = gate_k * B_v L A^2 L^{-1}  (per b,h),
so   y_b = sg * normed_b @ W_all_b,  W_all_b = sum_{k,h} Wq_{k,h} E^T W_{O,h}.
The queries never need to be materialized on device: phase 1 computes the
prefix Gram statistics (G, M, Cv) per (k, head) plus the centered/transposed
activations; the host does the 48x48 cholesky/solve/SVD algebra and folds
everything into a per-batch [D, D] effective matrix; phase 2 is one big
bf16 matmul  y = rstd * (Xc @ W_eff).

Sharding: core c -> batch b = c//2, head-half hh = c%2 (8 of 16 heads) for
phase 1; same core -> (batch, token-half) for phase 2.

All device matmuls run in bf16 (1 cycle/row on the PE vs 4 for fp32);
accumulation stays fp32 in PSUM. LN is folded: x is centered (exact, f32
stats) and cast to bf16 before the PE transpose; rstd is applied in the
projection epilogues; gamma is folded into the weights on the host.

Phase-1 schedule keeps the PE continuously fed (p-state ramps to 2.4 GHz
after 3 us of uninterrupted work): per prefix tile the transpose of tile
it+1 is interleaved with the projections of tile it; suffix-tile (non-
prefix) preprocessing runs after, overlapped with the shift/gram stages.
Shifted keys for the cross-covariance M are produced on the PE via a
shift matrix (SBUF->SBUF DMA serializes on one engine; partition-offset
matmul operands are illegal), and written back into the kvs tile so each
gram pair is a single [96, 256] matmul chain: [G | Cv | M'].
"""

import math

import numpy as np
import ml_dtypes

B, T, D = 4, 2048, 1024
H, HD = 16, 64
K_OPS, R = 4, 48
LN_EPS = 1e-5
NCORES = 8
HPC = H // 2            # heads per core = 8
NKQ = HPC * R           # 384 (per-core K width per op)
NV = HPC * HD           # 512
ND = D // 128           # 8 d-tiles
NTT = T // 128          # 16 token tiles
TH = T // 2             # phase-2 token half
# kvs per-head slot layout (width 512):
# [k0(0:48) k1(48:96) V(96:160) sh01(160:256) k2(256:304) k3(304:352)
#  Vd(352:416) sh23(416:512)]
KSLOT = [0, 48, 256, 304]
HW = 512

BF16 = ml_dtypes.bfloat16

_cache = {}


def _split_multi_waits(nc):
    """walrus codegen accepts at most one sync wait per instruction;
    move extra waits onto preceding wait-only NoOps on the same engine."""
    from concourse import mybir
    for fn in nc.m.functions:
        for bb in fn.blocks:
            insts = list(bb.instructions)
            new = []
            changed = False
            for inst in insts:
                si = inst.sync_info
                if si is not None and si.on_wait and len(si.on_wait) > 1:
                    waits = list(si.on_wait)
                    for j, w in enumerate(waits[:-1]):
                        new.append(mybir.InstNoOp(
                            name=f"{inst.name}-ws{j}", engine=inst.engine,
                            ins=[], outs=[],
                            sync_info=mybir.SyncInfo(on_wait=[w], on_update=[])))
                    inst.sync_info = mybir.SyncInfo(on_wait=[waits[-1]],
                                                    on_update=list(si.on_update))
                    changed = True
                new.append(inst)
            if changed:
                bb.instructions = new
    return nc


def _build_phase1(pl: int):
    import concourse.bass as bass
    import concourse.tile as tile
    from concourse import mybir
    from concourse.masks import make_identity
    from contextlib import ExitStack

    f32 = mybir.dt.float32
    bf16 = mybir.dt.bfloat16
    nc = bass.Bass()

    n_pt = (pl + 127) // 128     # prefix tiles (pl <= T-1 so n_pt <= NTT)
    nb = n_pt - 1                # tile-boundary count for the shifted gram

    xb = nc.dram_tensor("xb", [T, D], f32, kind="ExternalInput")
    wk = nc.dram_tensor("wk", [K_OPS, D, NKQ], bf16, kind="ExternalInput")
    wv = nc.dram_tensor("wv", [D, NV], bf16, kind="ExternalInput")
    xct_out = nc.dram_tensor("xct_out", [ND, 128, T], bf16, kind="ExternalOutput")
    rstd_out = nc.dram_tensor("rstd_out", [128, NTT], f32, kind="ExternalOutput")
    gmc_out = nc.dram_tensor("gmc_out", [HPC, 96, 512], f32, kind="ExternalOutput")

    with tile.TileContext(nc) as tc, ExitStack() as ctx:
        const = ctx.enter_context(tc.tile_pool(name="const", bufs=1))
        xch = ctx.enter_context(tc.tile_pool(name="xch", bufs=2))
        wkp = ctx.enter_context(tc.tile_pool(name="wkp", bufs=1))
        xctp = ctx.enter_context(tc.tile_pool(name="xctp", bufs=1))
        kvsp = ctx.enter_context(tc.tile_pool(name="kvsp", bufs=1))
        xcp = ctx.enter_context(tc.tile_pool(name="xcp", bufs=3))
        junkp = ctx.enter_context(tc.tile_pool(name="junkp", bufs=2))
        gstp = ctx.enter_context(tc.tile_pool(name="gstp", bufs=2))

        ident = const.tile([128, 128], bf16)
        make_identity(nc, ident)
        # shift matrix: S[t, j] = 1 iff t == j + 1, so (S^T K)[j] = K[j+1]
        shmat = const.tile([128, 128], bf16)
        nc.gpsimd.memset(shmat, 0.0)
        nc.gpsimd.affine_select(out=shmat, in_=shmat,
                                compare_op=mybir.AluOpType.not_equal,
                                fill=1.0, base=-1, channel_multiplier=1,
                                pattern=[[-1, 128]])
        eps_t = const.tile([128, 1], f32)
        nc.vector.memset(eps_t, LN_EPS)

        s_all = const.tile([128, NTT], f32)
        q_all = const.tile([128, NTT], f32)
        mneg_all = const.tile([128, NTT], f32)
        var_all = const.tile([128, NTT], f32)
        std_all = const.tile([128, NTT], f32)
        rstd_all = const.tile([128, NTT], f32)
        t1_all = const.tile([128, NTT], f32)

        # xcT in halves so the low half can stream out to DRAM early
        xcT_lo = xctp.tile([128, ND, TH], bf16, tag="lo")
        xcT_hi = xctp.tile([128, ND, TH], bf16, tag="hi")

        def xcT(it):
            # (tile, column slice) for token tile it
            half, off = divmod(it, 8)
            t = xcT_lo if half == 0 else xcT_hi
            return t, slice(off * 128, (off + 1) * 128)

        kvs = kvsp.tile([128, n_pt, HPC, HW], bf16)

        # x arrives in 2-tile chunks so work starts after the first 1 MB
        xchunks = {}

        def load_chunk(c):
            xt = xch.tile([128, 2, D], f32, tag="xch")
            nc.sync.dma_start(
                out=xt,
                in_=xb[c * 256:(c + 1) * 256, :].rearrange("(a p) n -> p a n", p=128))
            xchunks[c] = xt

        def prep_tile(it, tp_ps):
            if it // 2 not in xchunks:
                load_chunk(it // 2)
            xt = xchunks[it // 2][:, it % 2, :]
            c0, c1 = it, it + 1
            junk = junkp.tile([128, D], bf16)
            nc.scalar.activation(out=junk, in_=xt,
                                 func=mybir.ActivationFunctionType.Square,
                                 accum_out=q_all[:, c0:c1])
            nc.vector.tensor_reduce(out=s_all[:, c0:c1], in_=xt,
                                    axis=mybir.AxisListType.X,
                                    op=mybir.AluOpType.add)
            nc.vector.tensor_scalar_mul(mneg_all[:, c0:c1], s_all[:, c0:c1],
                                        -1.0 / D)
            nc.vector.tensor_mul(t1_all[:, c0:c1], s_all[:, c0:c1],
                                 s_all[:, c0:c1])
            nc.vector.tensor_scalar_mul(t1_all[:, c0:c1], t1_all[:, c0:c1],
                                        1.0 / D)
            nc.vector.tensor_sub(var_all[:, c0:c1], q_all[:, c0:c1],
                                 t1_all[:, c0:c1])
            nc.scalar.activation(out=std_all[:, c0:c1], in_=var_all[:, c0:c1],
                                 func=mybir.ActivationFunctionType.Sqrt,
                                 bias=eps_t[:, 0:1], scale=1.0 / D)
            nc.vector.reciprocal(rstd_all[:, c0:c1], std_all[:, c0:c1])
            xc = xcp.tile([128, D], bf16, tag="xc")
            nc.scalar.activation(out=xc, in_=xt,
                                 func=mybir.ActivationFunctionType.Identity,
                                 bias=mneg_all[:, c0:c1], scale=1.0)
            tp = tp_ps.tile([128, D], bf16)
            for d in range(ND):
                nc.tensor.transpose(tp[:, d * 128:(d + 1) * 128],
                                    xc[:, d * 128:(d + 1) * 128], ident)
            xt_t, xt_sl = xcT(it)
            nc.vector.tensor_copy(
                out=xt_t[:, :, xt_sl],
                in_=tp.rearrange("p (a n) -> p a n", a=ND))

        def proj_tile(it, proj_ps, vproj_ps):
            xt_t, tsl = xcT(it)
            for k in range(K_OPS):
                kp = proj_ps.tile([128, NKQ], f32)
                for d in range(ND):
                    nc.tensor.matmul(kp, xt_t[:, d, tsl],
                                     wk_sb[:, d, k * NKQ:(k + 1) * NKQ],
                                     start=(d == 0), stop=(d == ND - 1))
                ks = KSLOT[k]
                nc.scalar.activation(
                    out=kvs[:, it, :, ks:ks + R],
                    in_=kp.rearrange("p (h r) -> p h r", h=HPC),
                    func=mybir.ActivationFunctionType.Copy,
                    bias=0.0, scale=rstd_all[:, it:it + 1])
            vp = vproj_ps.tile([128, NV], f32)
            for d in range(ND):
                nc.tensor.matmul(vp, xt_t[:, d, tsl], wv_sb[:, d, :],
                                 start=(d == 0), stop=(d == ND - 1))
            nc.scalar.activation(
                out=kvs[:, it, :, 96:160],
                in_=vp.rearrange("p (h v) -> p h v", h=HPC),
                func=mybir.ActivationFunctionType.Copy,
                bias=0.0, scale=rstd_all[:, it:it + 1])
            # duplicate V slot for the pair23 gram pack (already scaled)
            nc.vector.tensor_copy(out=kvs[:, it, :, 352:416],
                                  in_=kvs[:, it, :, 96:160])
            if it == n_pt - 1 and pl < n_pt * 128:
                # zero keys/values of padded tokens so grams stay exact
                nc.vector.memset(kvs[pl - (n_pt - 1) * 128:128, it, :, :], 0.0)

        # first x chunk before the weights so tile-0 prep starts earliest
        load_chunk(0)
        wk_sb = wkp.tile([128, ND, K_OPS * NKQ], bf16)
        for k in range(K_OPS):
            nc.sync.dma_start(
                out=wk_sb[:, :, k * NKQ:(k + 1) * NKQ],
                in_=wk[k].rearrange("(a p) n -> p a n", p=128))
        wv_sb = wkp.tile([128, ND, NV], bf16)
        nc.sync.dma_start(out=wv_sb, in_=wv.rearrange("(a p) n -> p a n", p=128))

        # interleave prep(it) with proj(it-1) to keep the PE continuously fed
        with tc.tile_pool(name="tp_ps", bufs=2, space="PSUM") as tp_ps, \
             tc.tile_pool(name="warm_ps", bufs=1, space="PSUM") as warm_ps, \
             tc.tile_pool(name="proj_ps", bufs=2, space="PSUM") as proj_ps, \
             tc.tile_pool(name="vproj_ps", bufs=2, space="PSUM") as vproj_ps:
            # warm the PE p-state while the first x chunk is in flight;
            # small matmuls so the spin self-limits as the clock ramps
            wps = warm_ps.tile([128, 128], f32)
            for _ in range(36):
                nc.tensor.matmul(wps, ident, ident, start=True, stop=True)
            for it in range(NTT):
                prep_tile(it, tp_ps)
                if 1 <= it <= n_pt:
                    proj_tile(it - 1, proj_ps, vproj_ps)
                if it == 7:
                    # low half of xcT is complete: stream it out early
                    for d in range(ND):
                        nc.sync.dma_start(out=xct_out[d, :, 0:TH],
                                          in_=xcT_lo[:, d, :])
            if n_pt == NTT:
                proj_tile(n_pt - 1, proj_ps, vproj_ps)

        # ---- shifted keys on the PE, written back into kvs sh-slots ----
        with tc.tile_pool(name="sh_ps", bufs=4, space="PSUM") as sh_ps:
            for it in range(n_pt):
                for g, (ssl, dsl) in enumerate(
                        [(slice(0, 96), slice(160, 256)),
                         (slice(256, 352), slice(416, 512))]):
                    for hf in range(2):
                        hsl = slice(hf * 4, (hf + 1) * 4)
                        sp = sh_ps.tile([128, 4 * 96], f32)
                        nc.tensor.matmul(
                            sp.rearrange("p (h n) -> p h n", h=4), shmat,
                            kvs[:, it, hsl, ssl], start=True, stop=True)
                        nc.vector.tensor_copy(
                            out=kvs[:, it, hsl, dsl],
                            in_=sp.rearrange("p (h n) -> p h n", h=4))

        # boundary rows for M: row 127 of tile it (B1) x row 0 of tile it+1
        if nb > 0:
            bp = ctx.enter_context(tc.tile_pool(name="bp", bufs=1))
            b1 = bp.tile([nb, HPC, 2, 96], bf16)
            b0 = bp.tile([nb, HPC, 2, 96], bf16)
            nc.sync.dma_start(out=b1[:, :, 0, :], in_=kvs[127:128, 0:nb, :, 0:96])
            nc.sync.dma_start(out=b1[:, :, 1, :], in_=kvs[127:128, 0:nb, :, 256:352])
            nc.sync.dma_start(out=b0[:, :, 0, :], in_=kvs[0:1, 1:n_pt, :, 0:96])
            nc.sync.dma_start(out=b0[:, :, 1, :], in_=kvs[0:1, 1:n_pt, :, 256:352])

        # ---- packed grams: one [96, 256] chain per k-pair ----
        # out cols [0:96] = G blocks, [96:160] = Cv^T, [160:256] = M'
        with tc.tile_pool(name="gram_ps", bufs=2, space="PSUM") as gram_ps:
            for h in range(HPC):
                psa = gram_ps.tile([96, 256], f32, tag="a")
                psb = gram_ps.tile([96, 256], f32, tag="b")
                for it in range(n_pt):
                    last = (it == n_pt - 1)
                    nc.tensor.matmul(psa, kvs[:, it, h, 0:96],
                                     kvs[:, it, h, 0:256],
                                     start=(it == 0), stop=(last and nb == 0))
                    nc.tensor.matmul(psb, kvs[:, it, h, 256:352],
                                     kvs[:, it, h, 256:512],
                                     start=(it == 0), stop=(last and nb == 0))
                if nb > 0:
                    nc.tensor.matmul(psa[:, 160:256], b1[:, h, 0, :],
                                     b0[:, h, 0, :], start=False, stop=True,
                                     skip_group_check=True)
                    nc.tensor.matmul(psb[:, 160:256], b1[:, h, 1, :],
                                     b0[:, h, 1, :], start=False, stop=True,
                                     skip_group_check=True)
                gst = gstp.tile([96, 512], f32, tag="gst")
                nc.vector.tensor_copy(out=gst[:, 0:256], in_=psa)
                nc.vector.tensor_copy(out=gst[:, 256:512], in_=psb)
                nc.sync.dma_start(out=gmc_out[h], in_=gst)

        nc.sync.dma_start(out=rstd_out[:, :], in_=rstd_all)
        for d in range(ND):
            nc.sync.dma_start(out=xct_out[d, :, TH:T], in_=xcT_hi[:, d, :])
    return _split_multi_waits(nc)


def _build_phase2():
    import concourse.bass as bass
    import concourse.tile as tile
    from concourse import mybir
    from contextlib import ExitStack

    f32 = mybir.dt.float32
    bf16 = mybir.dt.bfloat16
    nc = bass.Bass()
    xct = nc.dram_tensor("xct", [ND, 128, TH], bf16, kind="ExternalInput")
    weff = nc.dram_tensor("weff", [ND, 128, D], bf16, kind="ExternalInput")
    rstd = nc.dram_tensor("rstd", [128, TH // 128], f32, kind="ExternalInput")
    y_out = nc.dram_tensor("y_out", [TH // 128, 128, D], f32, kind="ExternalOutput")

    nth = TH // 128  # 8 token tiles

    with tile.TileContext(nc) as tc, ExitStack() as ctx:
        const = ctx.enter_context(tc.tile_pool(name="const", bufs=1))
        ystp = ctx.enter_context(tc.tile_pool(name="ystp", bufs=3))
        y_ps = ctx.enter_context(tc.tile_pool(name="y_ps", bufs=2, space="PSUM"))

        # per-d tiles so the first matmuls start as soon as d0 lands
        xct_d, weff_d = [], []
        for d in range(ND):
            xt = const.tile([128, TH], bf16, tag=f"x{d}")
            nc.sync.dma_start(out=xt, in_=xct[d])
            wt = const.tile([128, D], bf16, tag=f"w{d}")
            nc.sync.dma_start(out=wt, in_=weff[d])
            xct_d.append(xt)
            weff_d.append(wt)
        rstd_sb = const.tile([128, nth], f32)
        nc.sync.dma_start(out=rstd_sb, in_=rstd[:, :])

        # warm the PE p-state while the loads are in flight
        warm_sb = const.tile([128, 128], bf16)
        nc.vector.memset(warm_sb, 0.0)
        with tc.tile_pool(name="warm_ps", bufs=1, space="PSUM") as warm_ps:
            wps = warm_ps.tile([128, 128], f32)
            for _ in range(72):
                nc.tensor.matmul(wps, warm_sb, warm_sb, start=True, stop=True)

        for tt in range(nth):
            tsl = slice(tt * 128, (tt + 1) * 128)
            yp = y_ps.tile([128, D], f32)
            for d in range(ND):
                nc.tensor.matmul(yp[:, 0:512], xct_d[d][:, tsl],
                                 weff_d[d][:, 0:512],
                                 start=(d == 0), stop=(d == ND - 1))
                nc.tensor.matmul(yp[:, 512:1024], xct_d[d][:, tsl],
                                 weff_d[d][:, 512:1024],
                                 start=(d == 0), stop=(d == ND - 1))
            y_sb = ystp.tile([128, D], f32, tag="y_sb")
            nc.scalar.activation(out=y_sb, in_=yp,
                                 func=mybir.ActivationFunctionType.Copy,
                                 bias=0.0, scale=rstd_sb[:, tt:tt + 1])
            nc.gpsimd.dma_start(out=y_out[tt], in_=y_sb)
    return _split_multi_waits(nc)


LAST_PERF = {}


def _numpy_fallback(hidden_states, W_K_ops, W_Q_ops, W_V, W_O, ln_gamma, ln_beta,
                    gate_alphas, gate_alpha, log_ridges, log_gammas, pl):
    x = np.asarray(hidden_states, np.float64)
    mu = x.mean(-1, keepdims=True)
    var = x.var(-1, keepdims=True)
    normed = (x - mu) / np.sqrt(var + LN_EPS) * ln_gamma + ln_beta
    values = (normed @ W_V).reshape(B, T, H, HD).transpose(0, 2, 1, 3)
    acc = np.zeros((B, H, T, HD))
    eye = np.eye(R)
    for k in range(K_OPS):
        ridge = math.exp(float(log_ridges[k]))
        gamma = math.exp(float(log_gammas[k]))
        gate = 1.0 / (1.0 + math.exp(-float(gate_alphas[k])))
        keys = (normed @ W_K_ops[k]).reshape(B, T, H, R).transpose(0, 2, 1, 3)
        qs = (normed @ W_Q_ops[k]).reshape(B, T, H, R).transpose(0, 2, 1, 3)
        pk = keys[:, :, :pl, :]
        G = np.einsum('bhlr,bhls->bhrs', pk, pk) + ridge * eye
        M = np.einsum('bhlr,bhls->bhrs', pk[:, :, 1:, :], pk[:, :, :-1, :])
        L = np.linalg.cholesky(G)
        Linv = np.linalg.inv(L)
        A = Linv @ M @ np.swapaxes(Linv, -1, -2)
        sig = np.linalg.svd(A, compute_uv=False)[..., 0]
        sig = np.maximum(sig, 1e-8)
        scale = min(gamma, 1.0) / np.maximum(sig, 1.0)
        A = A * scale[..., None, None]
        pv = values[:, :, :pl, :]
        Cv = np.einsum('bhld,bhlr->bhdr', pv, pk)
        Ginv = np.swapaxes(Linv, -1, -2) @ Linv
        Bv = Cv @ Ginv
        E = Bv @ L @ A @ A @ Linv
        out_k = np.einsum('bhdr,bhtr->bhtd', E, qs)
        acc = acc + gate * out_k
    out = acc.transpose(0, 2, 1, 3).reshape(B, T, H * HD) @ W_O
    sg = 1.0 / (1.0 + math.exp(-float(np.asarray(gate_alpha).ravel()[0])))
    return (sg * out).astype(np.float32)


def _decode_gmc(blk, k):
    """blk: [96, 512] device gram pack for one head; returns (G, CvT, Mp)."""
    pr, sub = divmod(k, 2)
    base = 256 * pr
    rsl = slice(sub * R, (sub + 1) * R)
    G = blk[rsl, base + sub * R:base + (sub + 1) * R]
    CvT = blk[rsl, base + 96:base + 160]
    Mp = blk[rsl, base + 160 + sub * R:base + 160 + (sub + 1) * R]
    return G, CvT, Mp


def kernel(hidden_states, W_K_ops, W_Q_ops, W_V, W_O, ln_gamma, ln_beta,
           gate_alphas, gate_alpha, log_ridges, log_gammas, prefix_len):
    from concourse.bass_utils import run_bass_kernel_spmd

    hidden_states = np.ascontiguousarray(np.asarray(hidden_states, np.float32))
    W_K_ops = np.asarray(W_K_ops, np.float32)
    W_Q_ops = np.asarray(W_Q_ops, np.float32)
    W_V = np.asarray(W_V, np.float32)
    W_O = np.asarray(W_O, np.float32)
    ln_gamma = np.asarray(ln_gamma, np.float32)
    ln_beta = np.asarray(ln_beta, np.float32)
    gate_alphas = np.asarray(gate_alphas, np.float32)
    log_ridges = np.asarray(log_ridges, np.float32)
    log_gammas = np.asarray(log_gammas, np.float32)
    pl = max(1, min(int(prefix_len), T - 1))

    if np.any(ln_beta != 0) or pl < 2:
        return _numpy_fallback(hidden_states, W_K_ops, W_Q_ops, W_V, W_O,
                               ln_gamma, ln_beta, gate_alphas, gate_alpha,
                               log_ridges, log_gammas, pl)

    # fold LN gamma into the projection weights; cast to bf16 for the device
    wk_f = (W_K_ops * ln_gamma[None, :, None]).astype(BF16)
    wv_f = (W_V * ln_gamma[:, None]).astype(BF16)

    wk_arr, wv_arr = [], []
    for hh in range(2):
        h0 = hh * HPC
        wk_arr.append(np.ascontiguousarray(wk_f[:, :, h0 * R:(h0 + HPC) * R]))
        wv_arr.append(np.ascontiguousarray(wv_f[:, h0 * HD:(h0 + HPC) * HD]))
    in1 = [{"xb": hidden_states[c // 2],
            "wk": wk_arr[c % 2], "wv": wv_arr[c % 2]} for c in range(NCORES)]

    key1 = ("p1", pl)
    if key1 not in _cache:
        _cache[key1] = _build_phase1(pl)
    r1 = run_bass_kernel_spmd(_cache[key1], in1, core_ids=list(range(NCORES)))
    LAST_PERF["p1"] = r1

    # ---- host: unpack G/M/Cv, 48x48 algebra, fold into per-batch W_eff ----
    ridge = np.exp(log_ridges.astype(np.float64))
    gamma_k = np.exp(log_gammas.astype(np.float64))
    gates = 1.0 / (1.0 + np.exp(-gate_alphas.astype(np.float64)))
    sg = 1.0 / (1.0 + math.exp(-float(np.asarray(gate_alpha).ravel()[0])))
    eye = np.eye(R)

    # E[b, k, h, HD, R]
    E = np.empty((B, K_OPS, H, HD, R), np.float64)
    for c in range(NCORES):
        b, h0 = c // 2, (c % 2) * HPC
        gmc = r1.results[c]["gmc_out"].astype(np.float64)  # [HPC, 96, 512]
        for hh in range(HPC):
            for k in range(K_OPS):
                G, CvT, Mp = _decode_gmc(gmc[hh], k)
                Gk = G + ridge[k] * eye
                M = Mp.T                      # M' = sum k_t k_{t+1}^T
                Cv = CvT.T                    # [HD, R]
                L = np.linalg.cholesky(Gk)
                Linv = np.linalg.inv(L)
                A = Linv @ M @ Linv.T
                sig = np.linalg.svd(A, compute_uv=False)[0]
                sig = max(sig, 1e-8)
                scale = min(gamma_k[k], 1.0) / max(sig, 1.0)
                A = A * scale
                Ginv = Linv.T @ Linv
                Bv = Cv @ Ginv
                E[b, k, h0 + hh] = gates[k] * (Bv @ L @ A @ A @ Linv)

    # W_eff[b] = sum_{k,h} (gamma o Wq_{k,h}) @ (E^T @ Wo_h), then * sg
    wq_f = (W_Q_ops * ln_gamma[None, :, None]).astype(np.float32)
    wq_flat = np.ascontiguousarray(
        wq_f.transpose(1, 0, 2).reshape(D, K_OPS * H * R))
    wo_r = W_O.reshape(H, HD, D)
    Et = np.ascontiguousarray(E.transpose(0, 1, 2, 4, 3).astype(np.float32))
    T1 = Et @ wo_r[None, None]               # [B, K, H, R, D]
    T1_flat = T1.reshape(B, K_OPS * H * R, D)
    weff_b = (wq_flat[None] @ T1_flat) * np.float32(sg)   # [B, D, D]

    if "p2" not in _cache:
        _cache["p2"] = _build_phase2()
    in2 = []
    for c in range(NCORES):
        b, hh = c // 2, c % 2
        xct_full = r1.results[c]["xct_out"]   # [ND, 128, T] bf16
        in2.append({
            "xct": np.ascontiguousarray(xct_full[:, :, hh * TH:(hh + 1) * TH]),
            "weff": np.ascontiguousarray(
                weff_b[b].astype(BF16).reshape(ND, 128, D)),
            "rstd": np.ascontiguousarray(
                r1.results[c]["rstd_out"][:, hh * 8:(hh + 1) * 8]),
        })
    r2 = run_bass_kernel_spmd(_cache["p2"], in2, core_ids=list(range(NCORES)))
    LAST_PERF["p2"] = r2

    y = np.empty((B, T, D), np.float32)
    for b in range(B):
        y[b, :TH] = r2.results[2 * b]["y_out"].reshape(TH, D)
        y[b, TH:] = r2.results[2 * b + 1]["y_out"].reshape(TH, D)
    return y
